# revision 40
# baseline (speedup 1.0000x reference)
"""Trainium2 Bass kernel for a 13-layer causal dilated conv stack with gating.

Model (per reference):
    Wx_f = 13 causal dilated convs (K=2, dilation 2^i) over x with Wf
    Wx_g = same with Wg
    out  = tanh(Wx_f + h@Vf) * sigmoid(Wx_g + h@Vg)

Shapes: x (16, 8192, 64) f32, h (16, 64), Wf/Wg (13, 2, 64, 64), Vf/Vg (64, 64).

Strategy (v2: fused layer pairs + Karatsuba; 202us -> 184us):
  - Data-parallel over batch: 2 batch elements per core on 8 cores, no
    collectives. On-chip layout [128 partitions = (b*64 + c), T free], fp16
    datapath, host pre/post transposes (as v1).
  - Layers are fused in PAIRS: layers (2k, 2k+1) compose into a single 4-tap
    conv with dilation d=4^k and host-precomputed product weights
    V0..V3 (y = x@V0 + x(-d)@V1 + x(-2d)@V2 + x(-3d)@V3). This halves the
    number of PSUM->SBUF activation materializations (the v1 co-bottleneck:
    ACT/DVE PSUM drains cost ~1.35ns/col vs PE 0.42ns/col).
  - Composite pairs 1-4 (both branches) use KARATSUBA over their z^(2d)
    structure: with P = V0 + V1 z^d, Q = V2 + V3 z^d and
    D2(t) = x(t) - x(t-2d),
      even cols (t mod 4d < 2d):   y = p - (Q D2),  p = ((P+Q) x)
      odd  cols (t2 = t1 + 2d):    y = p(t1) + (P D2)(t2)
    -> 6 matmul-cols per 2 output cols instead of 8: 25% less PE time.
    Per 1024-col supertile: p matmuls -> PSUM pp; -Q D2 -> PSUM pq; pq is
    bounced to SBUF fp16 (ACT; the BIR verifier forbids dual-PSUM
    TensorTensor), y-even = DVE add (pp + q16) to a strided dst; the P D2
    matmuls then ACCUMULATE into pp (after the DVE read) so y-odd is a
    plain PSUM copy (3:1 ACT:DVE round robin). r-matmuls lag one supertile
    so the DVE read of pp never stalls the PE.
  - D2 tensors: DVE and GpSimd alternate supertiles (fp16 TT sub runs 2x
    on DVE, ~0.66ns/col; GpSimd's software sub is ~2.1ns/col but otherwise
    idle), into THREE round-robin D buffers; each pair's D is emitted a
    full pair ahead (prefetched during the previous plan entry) so the
    ~2us GpSimd latency is off the critical path.
  - Pairs 0 and 5 are PLAIN 4-tap composites (4 accumulating matmuls +
    one drain): pair 0's 2-col interleave makes strided karatsuba writes
    fall off the DVE fast path, and karatsuba aux would overload ACT/DVE
    in the g-pair5 tail; plain also needs no D, decluttering the
    DMA-paced startup (pair 0 = the first phase).
  - Layer 12 (d=4096) stays un-fused (odd layer count): standard 2-matmul
    tiles. solo-f's drains ARE the tanh (ACT, fused h@Vf bias), and
    solo-f interleaves with the PE-heavy plain g-pair0 (its tanh drains
    own the ACT budget; g0's drains are forced to DVE).
  - Causality: 256-col zero margins cover reads for d<=64 taps; pair 4
    (d=256): D2's first 512 cols are a copy; pair 5 (d=1024) and solo
    skip tile-aligned out-of-range taps.
  - Tail: plain g-pair5 tiles interleave with solo-g + sigmoid/mul
    epilogue and progressive fp16 output DMA chunks; the last tile runs
    256-col chains (mm -> sigmoid -> mul -> DMA) to halve the serial
    suffix. Startup: fine x chunks + first-needed-first weight DMA splits;
    PE lane warm-ups + on-device h@V biases as v1.
"""

import sys

import numpy as np

for _p in ("/opt/trn_rl_repo",):
    if _p not in sys.path:
        sys.path.append(_p)

B, T, C = 16, 8192, 64
K = 2
NUM_LAYERS = 13
N_CORES = 8
BPC = B // N_CORES          # batch elements per core
P = 2 * C                   # partitions used: (b, c) pairs
MARGIN = 256                # causal zero margin
ST = 1024                   # supertile cols (pairs 0-4)
NST = T // ST               # 8
TILE = 512                  # tile cols (pair 5, solo, drains, psum width)
NT = T // TILE              # 16
PAIR_D = [1, 4, 16, 64, 256, 1024]   # first-layer dilation of pair k
NPAIR = 6
SOLO_D = 4096

# schedule: (kind, branch, pair_k), strictly sequential (f0's karatsuba is
# PE-paced beyond x chunk 0, so there is no DMA idle to fill with g0).
SCHED = ([("pair", "f", k) for k in range(6)] + [("solo", "f", None)]
         + [("pair", "g", k) for k in range(6)] + [("solo", "g", None)])

# Pairs are either KARATSUBA (6 weight tiles [A0,A1,Qn0,Qn1,P0,P1]) or
# PLAIN 4-tap (4 tiles [V0,V1,V2,V3]). Plain: both pair-0s (blk-2-strided
# y-even writes fall off the DVE fast path; plain needs no D at all, which
# also declutters the DMA-paced startup), both pair-5s (karatsuba aux
# overloads ACT/DVE in the tail) and g-pair0's merge with solo-f.
PLAIN = {("f", 0), ("g", 0), ("f", 5), ("g", 5)}
WBASE = {}
_w = 0
for _e in SCHED:
    WBASE[(_e[0], _e[1], _e[2])] = _w
    if _e[0] == "pair":
        _w += 4 if (_e[1], _e[2]) in PLAIN else 6
    else:
        _w += 2
NW = _w                     # 70

W_HEAD = 4                  # f-pair0's V0..V3: DMA'd before x chunk 0
W_MID = 16                  # through f2's weights

# x input chunks: fine early (first tiles start sooner), coarse late
XEDGE = [0, 512, 1024, 2048, 3072, 4096, 6144, 8192]
# output chunks, big early / small late so the final transfer is tiny
QEDGE = [0, 2048, 4096, 6144, 7168, 7680, 7936, 8192]
NQ = len(QEDGE) - 1

_PROGRAM_CACHE = {}


def _build_program():
    import concourse.bacc as bacc
    import concourse.tile as tile
    from concourse import mybir
    from concourse.ap import AP

    f32 = mybir.dt.float32
    fp16 = mybir.dt.float16
    AF = mybir.ActivationFunctionType

    nc = bacc.Bacc("TRN2", target_bir_lowering=False, debug=False)

    consts = nc.dram_tensor("consts", [P, 2 * P + 1], f32,
                            kind="ExternalInput").ap()
    xin = nc.dram_tensor("xr", [P, T], fp16, kind="ExternalInput").ap()
    wconv = nc.dram_tensor("wr", [P, NW * P], fp16, kind="ExternalInput").ap()
    outs = [nc.dram_tensor(f"out_q{q}", [P, QEDGE[q + 1] - QEDGE[q]], fp16,
                           kind="ExternalOutput").ap() for q in range(NQ)]

    with tile.TileContext(nc) as tc:
        with (
            tc.tile_pool(name="persist", bufs=1) as persist,
            tc.tile_pool(name="epool", bufs=4) as epool,
            tc.tile_pool(name="qpool", bufs=3) as qpool,
            tc.tile_pool(name="mpsum", bufs=6, space="PSUM") as mpsum,
            tc.tile_pool(name="p5psum", bufs=2, space="PSUM") as p5psum,
        ):
            # ---- persistent buffers ---------------------------------------
            x0 = persist.tile([P, MARGIN + T], fp16, name="x0", tag="x0")
            bufA = persist.tile([P, MARGIN + T], fp16, name="bufA", tag="bufA")
            bufB = persist.tile([P, MARGIN + T], fp16, name="bufB", tag="bufB")
            scratch = persist.tile([P, MARGIN + T], fp16, name="scratch",
                                   tag="scratch")
            D0 = persist.tile([P, MARGIN + T], fp16, name="D0", tag="D0")
            D1 = persist.tile([P, MARGIN + T], fp16, name="D1", tag="D1")
            D2 = persist.tile([P, MARGIN + T], fp16, name="D2", tag="D2")
            call = persist.tile([P, 2 * P + 1], f32, name="call", tag="call")
            wall = persist.tile([P, NW * P], fp16, name="wall", tag="wall")
            tanh16 = persist.tile([P, T], fp16, name="tanh16", tag="tanh16")
            out16 = persist.tile([P, T], fp16, name="out16", tag="out16")
            # only x0's margin gates the first matmul; the rest memset
            # on the otherwise-idle GpSimd so the DVE queue starts clear
            nc.vector.memset(x0[:, 0:MARGIN], 0.0)
            for buf in (bufA, bufB, scratch, D0, D1, D2):
                nc.gpsimd.memset(buf[:, 0:MARGIN], 0.0)

            # ---- input DMAs, ordered by first consumption -----------------
            nc.sync.dma_start(out=wall[:, 0:P], in_=wconv[:, 0:P])
            nc.sync.dma_start(
                out=x0[:, MARGIN + XEDGE[0]:MARGIN + XEDGE[1]],
                in_=xin[:, XEDGE[0]:XEDGE[1]])
            nc.sync.dma_start(out=wall[:, P:W_HEAD * P],
                              in_=wconv[:, P:W_HEAD * P])
            nc.sync.dma_start(out=call, in_=consts)
            # x chunks 1-2 land before f1/f2's weight block (startup is
            # DMA-bandwidth-paced; f1's weights aren't needed until ~25us)
            for xc in range(1, 3):
                nc.sync.dma_start(
                    out=x0[:, MARGIN + XEDGE[xc]:MARGIN + XEDGE[xc + 1]],
                    in_=xin[:, XEDGE[xc]:XEDGE[xc + 1]])
            nc.sync.dma_start(out=wall[:, W_HEAD * P:W_MID * P],
                              in_=wconv[:, W_HEAD * P:W_MID * P])
            for xc in range(3, len(XEDGE) - 1):
                nc.sync.dma_start(
                    out=x0[:, MARGIN + XEDGE[xc]:MARGIN + XEDGE[xc + 1]],
                    in_=xin[:, XEDGE[xc]:XEDGE[xc + 1]])
            nc.sync.dma_start(out=wall[:, W_MID * P:],
                              in_=wconv[:, W_MID * P:])

            # ---- PE lane warm-ups + h @ V biases --------------------------
            bias_ps = mpsum.tile([P, 8], f32, name="bias_ps", tag="mp")
            x0w = x0[:, MARGIN:MARGIN + 1]
            ww0 = wall[:, 0:1]
            ww1 = wall[:, W_HEAD * P:W_HEAD * P + 1]
            ww2 = wall[:, W_MID * P:W_MID * P + 1]
            nc.tensor.matmul(bias_ps[0:1, 4:5], lhsT=x0w, rhs=x0w,
                             start=True, stop=True)
            nc.tensor.matmul(bias_ps[0:1, 5:6], lhsT=ww0, rhs=ww0,
                             start=True, stop=True)
            nc.tensor.matmul(bias_ps[0:1, 6:7], lhsT=ww1, rhs=ww1,
                             start=True, stop=True)
            nc.tensor.matmul(bias_ps[0:1, 7:8], lhsT=ww2, rhs=ww2,
                             start=True, stop=True)
            h_t = call[:, 2 * P:2 * P + 1]
            bias = []
            for i in range(2):
                nc.tensor.matmul(bias_ps[:, i:i + 1],
                                 lhsT=call[:, i * P:(i + 1) * P],
                                 rhs=h_t, start=True, stop=True)
                bias_sb = persist.tile([P, 1], f32, name=f"bias{i}",
                                       tag=f"bias{i}")
                nc.vector.tensor_copy(bias_sb, bias_ps[:, i:i + 1])
                bias.append(bias_sb)

            # ---- helpers --------------------------------------------------
            def sap(buf, col, blkstride, nblk, blklen):
                """Strided AP: nblk blocks of blklen cols every blkstride."""
                base = buf[:, col:col + 1]
                return AP(base.tensor, base.offset,
                          [list(base.ap[0]), [blkstride, nblk], [1, blklen]])

            def wt(base, i):
                return wall[:, (base + i) * P:(base + i + 1) * P]

            ectr = [0]

            def emit_D(db, src, k, s):
                """D2 for pair k over supertile s into db, alternating
                DVE / GpSimd. Pair 4: tile 0 of supertile 0 is a copy
                (x(t-512) is beyond the margin)."""
                twod = 2 * PAIR_D[k]
                c0 = s * ST
                # 50:50 DVE:GpSimd — DVE is the k-phase bottleneck
                # (ye + D + yo share), GpSimd idles; its 2.2us latency is
                # hidden by the one-pair prefetch lead
                eng = nc.gpsimd if ectr[0] % 2 == 1 else nc.vector
                ectr[0] += 1
                dst = db[:, MARGIN + c0:MARGIN + c0 + ST]
                if twod <= MARGIN:
                    if eng is nc.vector:
                        # 2x512 halves: a D that falls off the DVE fast path
                        # (sporadic ~2us mode) blocks the queue half as long
                        for o in (0, TILE):
                            a = MARGIN + c0 + o
                            eng.tensor_sub(db[:, a:a + TILE],
                                           src[:, a:a + TILE],
                                           src[:, a - twod:a + TILE - twod])
                    else:
                        eng.tensor_sub(
                            dst, src[:, MARGIN + c0:MARGIN + c0 + ST],
                            src[:, MARGIN + c0 - twod:
                                MARGIN + c0 + ST - twod])
                else:  # pair 4, twod = 512
                    if s == 0:
                        eng.tensor_copy(db[:, MARGIN:MARGIN + TILE],
                                        src[:, MARGIN:MARGIN + TILE])
                        eng.tensor_sub(
                            db[:, MARGIN + TILE:MARGIN + ST],
                            src[:, MARGIN + TILE:MARGIN + ST],
                            src[:, MARGIN:MARGIN + TILE])
                    else:
                        eng.tensor_sub(dst,
                                       src[:, MARGIN + c0:MARGIN + c0 + ST],
                                       src[:, MARGIN + c0 - twod:
                                           MARGIN + c0 + ST - twod])

            # y-odd copies: 3:1 ACT:DVE (DVE also carries ye TTs + half of D)
            yo_rr = [0]

            def yo_drain():
                i = yo_rr[0] % 4
                yo_rr[0] += 1
                return nc.vector.tensor_copy if i == 3 else nc.scalar.copy

            # plain-pair drains: 1:1 ACT:DVE
            pl_rr = [0]

            def pl_drain():
                pl_rr[0] += 1
                return (nc.scalar.copy if pl_rr[0] % 2 == 0
                        else nc.vector.tensor_copy)

            def pair_pq(br, k, s, src, dst, db, wb):
                """p and q matmuls + y-even for supertile s; returns pp."""
                d = PAIR_D[k]
                twod, fourd = 2 * d, 4 * d
                c0 = s * ST
                nb = ST // fourd

                def ev(buf, off):
                    return sap(buf, MARGIN + c0 - off, fourd, nb, twod)

                pp = mpsum.tile([P, ST // 2], f32, name=f"pp_{br}{k}_{s}",
                                tag="mp")
                pq = mpsum.tile([P, ST // 2], f32, name=f"pq_{br}{k}_{s}",
                                tag="mp")
                nc.tensor.matmul(pp, lhsT=wt(wb, 0), rhs=ev(src, 0),
                                 start=True, stop=False)
                nc.tensor.matmul(pp, lhsT=wt(wb, 1), rhs=ev(src, d),
                                 start=False, stop=True)
                nc.tensor.matmul(pq, lhsT=wt(wb, 2), rhs=ev(db, 0),
                                 start=True, stop=False)
                nc.tensor.matmul(pq, lhsT=wt(wb, 3), rhs=ev(db, d),
                                 start=False, stop=True)
                # verifier forbids dual-PSUM TT: bounce q through SBUF,
                # then y-even = pp(PSUM) + q16(SBUF) on DVE; pp survives for
                # the r accumulation.
                q16 = qpool.tile([P, ST // 2], fp16, name=f"q16_{br}{k}_{s}",
                                 tag="q16")
                nc.scalar.copy(q16, pq)
                nc.vector.tensor_add(ev(dst, 0), pp, q16)
                return pp

            def pair_r(br, k, s, dst, db, wb, pp):
                """r matmuls accumulate into pp; y-odd copy."""
                d = PAIR_D[k]
                twod, fourd = 2 * d, 4 * d
                c0 = s * ST
                nb = ST // fourd

                def ev(buf, off):
                    return sap(buf, MARGIN + c0 - off, fourd, nb, twod)

                nc.tensor.matmul(pp, lhsT=wt(wb, 4), rhs=ev(db, -twod),
                                 start=False, stop=False,
                                 skip_group_check=True)
                nc.tensor.matmul(pp, lhsT=wt(wb, 5), rhs=ev(db, -twod + d),
                                 start=False, stop=True,
                                 skip_group_check=True)
                yo_drain()(ev(dst, -twod), pp)

            def plain_tile(br, k, j, src, dst, wb, drain_eng=None,
                           fine=False):
                """PLAIN 4-tap composite pair at 512-col tile granularity.
                Tap i reads src(t - i*d); margin covers small d, tile-aligned
                skips cover d=1024 (skip while (j+1)*TILE <= i*d)."""
                d = PAIR_D[k]
                c0 = j * TILE
                taps = [i for i in range(4) if (j + 1) * TILE > i * d]
                ps = p5psum.tile([P, TILE], f32, name=f"pl_{br}{k}_{j}",
                                 tag="p5")
                for n, i in enumerate(taps):
                    nc.tensor.matmul(
                        ps, lhsT=wt(wb, i),
                        rhs=src[:, MARGIN + c0 - i * d:
                                MARGIN + c0 - i * d + TILE],
                        start=(n == 0), stop=(n == len(taps) - 1))
                eng = drain_eng if drain_eng is not None else pl_drain()
                if fine:
                    eng(dst[:, MARGIN + c0:MARGIN + c0 + 256], ps[:, 0:256])
                    eng(dst[:, MARGIN + c0 + 256:MARGIN + c0 + TILE],
                        ps[:, 256:TILE])
                else:
                    eng(dst[:, MARGIN + c0:MARGIN + c0 + TILE], ps)

            def solo_tile(br, j, src, wb, fuse):
                """Solo layer 12 (d=4096): psum -> fuse(j, ps)."""
                c0 = j * TILE
                ps = mpsum.tile([P, TILE], f32, name=f"s12{br}_{j}", tag="mp")
                has0 = c0 >= SOLO_D
                nc.tensor.matmul(ps, lhsT=wt(wb, 0),
                                 rhs=src[:, MARGIN + c0:MARGIN + c0 + TILE],
                                 start=True, stop=not has0)
                if has0:
                    nc.tensor.matmul(
                        ps, lhsT=wt(wb, 1),
                        rhs=src[:, MARGIN + c0 - SOLO_D:
                                MARGIN + c0 - SOLO_D + TILE],
                        start=False, stop=True)
                fuse(j, ps)

            def tanh_drain(j, ps):
                t0 = j * TILE
                nc.scalar.activation(tanh16[:, t0:t0 + TILE], ps,
                                     AF.Tanh, bias=bias[0])

            nxt_q = [0]

            def flush_outputs(jj):
                while (nxt_q[0] < NQ
                       and QEDGE[nxt_q[0] + 1] <= (jj + 1) * TILE):
                    q = nxt_q[0]
                    nc.sync.dma_start(
                        out=outs[q], in_=out16[:, QEDGE[q]:QEDGE[q + 1]])
                    nxt_q[0] += 1

            def epilogue(j, ps):
                t0 = j * TILE
                sig = epool.tile([P, TILE], fp16, name=f"sig{j}", tag="sig")
                nc.scalar.activation(sig, ps, AF.Sigmoid, bias=bias[1])
                # alternate the gating mul DVE/GpSimd: the tail is aux-bound
                eng = nc.vector if j % 2 == 0 else nc.gpsimd
                eng.tensor_mul(out16[:, t0:t0 + TILE],
                               tanh16[:, t0:t0 + TILE], sig)
                flush_outputs(j)

            # ---- plan -----------------------------------------------------
            # chains: f: x0->A->B->A->B->A->B ->tanh16; g: x0->scratch->A->..
            fchain = [x0, bufA, bufB, bufA, bufB, bufA, bufB]
            gchain = [x0, scratch, bufA, bufB, bufA, bufB, bufA]
            plan = []
            for k in range(6):
                plan.append(("pair", "f", k, fchain[k], fchain[k + 1]))
            plan.append(("solo", "f", None, fchain[6], None))
            for k in range(6):
                plan.append(("pair", "g", k, gchain[k], gchain[k + 1]))
            plan.append(("solo", "g", None, gchain[6], None))

            # D buffers ping-pong over karatsuba pairs in plan order; each
            # pair's D is prefetched during the PREVIOUS plan entry (g1's
            # during the merged solo-f + g0 phase).
            kpairs = [e for e in plan
                      if e[0] == "pair" and (e[1], e[2]) not in PLAIN]
            dbuf_of = {}
            for n, e in enumerate(kpairs):
                dbuf_of[(e[1], e[2])] = (D0, D1, D2)[n % 3]

            def prefetch_D(i, s):
                """During plan entry i, emit supertile s of the next
                karatsuba pair's D (if entry i+1 is one)."""
                if i + 1 < len(plan):
                    e = plan[i + 1]
                    if e[0] == "pair" and (e[1], e[2]) not in PLAIN:
                        emit_D(dbuf_of[(e[1], e[2])], e[3], e[2], s)

            for i, e in enumerate(plan):
                kind, br, k, src, dst = e
                if kind == "pair" and (br, k) not in PLAIN:
                    db = dbuf_of[(br, k)]
                    wb = WBASE[("pair", br, k)]
                    prev = None
                    for s in range(NST):
                        pp = pair_pq(br, k, s, src, dst, db, wb)
                        if prev is not None:
                            pair_r(br, k, s - 1, dst, db, wb, prev)
                            prefetch_D(i, s - 1)
                        prev = pp
                    pair_r(br, k, NST - 1, dst, db, wb, prev)
                    prefetch_D(i, NST - 1)
                elif kind == "pair" and not (br == "g" and k == 0):
                    # plain pairs run standalone (g-pair0 merges with solo-f)
                    wb = WBASE[("pair", br, k)]
                    if br == "g":
                        # tail: interleave solo-g + epilogue per tile
                        wbs = WBASE[("solo", "g", None)]
                        for j in range(NT):
                            plain_tile(br, 5, j, src, dst, wb,
                                       drain_eng=nc.vector.tensor_copy,
                                       fine=(j == NT - 1))
                            if j >= 1:
                                solo_tile("g", j - 1, dst, wbs, epilogue)
                        # last tile: 256-col chains so the serial suffix
                        # (mm -> sigmoid -> mul -> DMA) halves
                        jf = NT - 1
                        for h in (0, 1):
                            o = jf * TILE + h * 256
                            psf = mpsum.tile([P, 256], f32,
                                             name=f"s12gf{h}", tag="mp")
                            nc.tensor.matmul(
                                psf, lhsT=wt(wbs, 0),
                                rhs=dst[:, MARGIN + o:MARGIN + o + 256],
                                start=True, stop=False)
                            nc.tensor.matmul(
                                psf, lhsT=wt(wbs, 1),
                                rhs=dst[:, MARGIN + o - SOLO_D:
                                        MARGIN + o - SOLO_D + 256],
                                start=False, stop=True)
                            sigf = epool.tile([P, 256], fp16,
                                              name=f"sigf{h}", tag="sig")
                            nc.scalar.activation(sigf, psf, AF.Sigmoid,
                                                 bias=bias[1])
                            nc.vector.tensor_mul(out16[:, o:o + 256],
                                                 tanh16[:, o:o + 256], sigf)
                            nc.sync.dma_start(out=outs[NQ - 2 + h],
                                              in_=out16[:, o:o + 256])
                    else:
                        for j in range(NT):
                            plain_tile(br, k, j, src, dst, wb)
                            if j % 2 == 1:
                                prefetch_D(i, j // 2)
                elif kind == "solo" and br == "f":
                    # merged phase: solo-f's tanh drains own ACT, so the
                    # PE-heavy plain g-pair0 interleaves here (g0's drains
                    # forced to DVE); prefetches g1's D from scratch.
                    wb_s = WBASE[("solo", "f", None)]
                    eg = plan[i + 1]
                    assert eg[:3] == ("pair", "g", 0)
                    wb_g0 = WBASE[("pair", "g", 0)]
                    for j in range(NT):
                        solo_tile("f", j, src, wb_s, tanh_drain)
                        plain_tile("g", 0, j, eg[3], eg[4], wb_g0,
                                   drain_eng=nc.vector.tensor_copy)
                        if j % 2 == 1:
                            prefetch_D(i + 1, j // 2)
                # g-pair0 handled in the merged phase; solo-g in the tail

    nc.compile()
    return nc


def get_program():
    if "nc" not in _PROGRAM_CACHE:
        _PROGRAM_CACHE["nc"] = _build_program()
    return _PROGRAM_CACHE["nc"]


def make_in_maps(x, h, Wf, Wg, Vf, Vg):
    x = np.asarray(x, dtype=np.float32)
    h = np.asarray(h, dtype=np.float32)
    eye2 = np.eye(2, dtype=np.float64)
    Wn = {"f": np.asarray(Wf, dtype=np.float64),
          "g": np.asarray(Wg, dtype=np.float64)}
    wpack = np.zeros((NW, P, P), dtype=np.float32)
    for (kind, br, k), base in WBASE.items():
        Wb = Wn[br]
        if kind == "pair":
            W0a, W1a = Wb[2 * k, 0], Wb[2 * k, 1]
            W0b, W1b = Wb[2 * k + 1, 0], Wb[2 * k + 1, 1]
            V0 = W1a @ W1b
            V1 = W0a @ W1b
            V2 = W1a @ W0b
            V3 = W0a @ W0b
            if (br, k) in PLAIN:    # plain 4-tap
                mats = [V0, V1, V2, V3]
            else:                   # karatsuba
                mats = [V0 + V2, V1 + V3, -V2, -V3, V0, V1]
        else:
            mats = [Wb[12, 1], Wb[12, 0]]
        for i, m in enumerate(mats):
            wpack[base + i] = np.kron(eye2, m).astype(np.float32)
    wcols = wpack.transpose(1, 0, 2).reshape(P, NW * P).astype(np.float16)
    vcat = np.concatenate(
        [np.kron(np.eye(2, dtype=np.float32), np.asarray(V, dtype=np.float32))
         for V in (Vf, Vg)], axis=1)  # [128, 256]

    in_maps = []
    for core in range(N_CORES):
        sl = slice(core * BPC, (core + 1) * BPC)
        xcm = x[sl].transpose(0, 2, 1).reshape(P, T) \
            .astype(np.float16)  # [(b,c), t]
        consts = np.ascontiguousarray(
            np.concatenate([vcat, h[sl].reshape(P, 1)], axis=1))
        in_maps.append({"consts": consts, "xr": xcm, "wr": wcols})
    return in_maps


def _to_f32(a):
    a = np.asarray(a)
    if a.dtype in (np.float32, np.float16):
        return a.astype(np.float32)
    u = a.view(np.uint16).astype(np.uint32) << np.uint32(16)
    return u.view(np.float32)


def assemble_output(results):
    full = np.empty((B, T, C), dtype=np.float32)
    for core, r in enumerate(results):
        cm = np.concatenate(
            [_to_f32(r[f"out_q{q}"]) for q in range(NQ)], axis=1)
        full[core * BPC:(core + 1) * BPC] = \
            cm.reshape(BPC, C, T).transpose(0, 2, 1)
    return full


def kernel(x, h, Wf, Wg, Vf, Vg):
    from concourse import bass_utils

    nc = get_program()
    in_maps = make_in_maps(x, h, Wf, Wg, Vf, Vg)
    res = bass_utils.run_bass_kernel_spmd(nc, in_maps,
                                          core_ids=list(range(N_CORES)))
    return assemble_output(res.results)


# revision 41
# speedup vs baseline: 1.0765x; 1.0765x over previous
"""Trainium2 Bass kernel for a 13-layer causal dilated conv stack with gating.

Model (per reference):
    Wx_f = 13 causal dilated convs (K=2, dilation 2^i) over x with Wf
    Wx_g = same with Wg
    out  = tanh(Wx_f + h@Vf) * sigmoid(Wx_g + h@Vg)

Shapes: x (16, 8192, 64) f32, h (16, 64), Wf/Wg (13, 2, 64, 64), Vf/Vg (64, 64).

Strategy (v2: fused layer pairs + Karatsuba; 202us -> 184us):
  - Data-parallel over batch: 2 batch elements per core on 8 cores, no
    collectives. On-chip layout [128 partitions = (b*64 + c), T free], fp16
    datapath, host pre/post transposes (as v1).
  - Layers are fused in PAIRS: layers (2k, 2k+1) compose into a single 4-tap
    conv with dilation d=4^k and host-precomputed product weights
    V0..V3 (y = x@V0 + x(-d)@V1 + x(-2d)@V2 + x(-3d)@V3). This halves the
    number of PSUM->SBUF activation materializations (the v1 co-bottleneck:
    ACT/DVE PSUM drains cost ~1.35ns/col vs PE 0.42ns/col).
  - Composite pairs 1-4 (both branches) use KARATSUBA over their z^(2d)
    structure: with P = V0 + V1 z^d, Q = V2 + V3 z^d and
    D2(t) = x(t) - x(t-2d),
      even cols (t mod 4d < 2d):   y = p - (Q D2),  p = ((P+Q) x)
      odd  cols (t2 = t1 + 2d):    y = p(t1) + (P D2)(t2)
    -> 6 matmul-cols per 2 output cols instead of 8: 25% less PE time.
    Per 1024-col supertile: p matmuls -> PSUM pp; -Q D2 -> PSUM pq; pq is
    bounced to SBUF fp16 (ACT; the BIR verifier forbids dual-PSUM
    TensorTensor), y-even = DVE add (pp + q16) to a strided dst; the P D2
    matmuls then ACCUMULATE into pp (after the DVE read) so y-odd is a
    plain PSUM copy (3:1 ACT:DVE round robin). r-matmuls lag one supertile
    so the DVE read of pp never stalls the PE.
  - D2 tensors: DVE and GpSimd alternate supertiles (fp16 TT sub runs 2x
    on DVE, ~0.66ns/col; GpSimd's software sub is ~2.1ns/col but otherwise
    idle), into THREE round-robin D buffers; each pair's D is emitted a
    full pair ahead (prefetched during the previous plan entry) so the
    ~2us GpSimd latency is off the critical path.
  - Pairs 0 and 5 are PLAIN 4-tap composites (4 accumulating matmuls +
    one drain): pair 0's 2-col interleave makes strided karatsuba writes
    fall off the DVE fast path, and karatsuba aux would overload ACT/DVE
    in the g-pair5 tail; plain also needs no D, decluttering the
    DMA-paced startup (pair 0 = the first phase).
  - Layer 12 (d=4096) stays un-fused (odd layer count): standard 2-matmul
    tiles. solo-f's drains ARE the tanh (ACT, fused h@Vf bias), and
    solo-f interleaves with the PE-heavy plain g-pair0 (its tanh drains
    own the ACT budget; g0's drains are forced to DVE).
  - Causality: 256-col zero margins cover reads for d<=64 taps; pair 4
    (d=256): D2's first 512 cols are a copy; pair 5 (d=1024) and solo
    skip tile-aligned out-of-range taps.
  - Tail: plain g-pair5 tiles interleave with solo-g + sigmoid/mul
    epilogue and progressive fp16 output DMA chunks; the last tile runs
    256-col chains (mm -> sigmoid -> mul -> DMA) to halve the serial
    suffix. Startup: fine x chunks + first-needed-first weight DMA splits;
    PE lane warm-ups + on-device h@V biases as v1.
"""

import sys

import numpy as np

for _p in ("/opt/trn_rl_repo",):
    if _p not in sys.path:
        sys.path.append(_p)

B, T, C = 16, 8192, 64
K = 2
NUM_LAYERS = 13
N_CORES = 8
BPC = B // N_CORES          # batch elements per core
P = 2 * C                   # partitions used: (b, c) pairs
MARGIN = 256                # causal zero margin
ST = 1024                   # supertile cols (pairs 0-4)
NST = T // ST               # 8
TILE = 512                  # tile cols (pair 5, solo, drains, psum width)
NT = T // TILE              # 16
PAIR_D = [1, 4, 16, 64, 256, 1024]   # first-layer dilation of pair k
NPAIR = 6
SOLO_D = 4096

# schedule: (kind, branch, pair_k), strictly sequential (f0's karatsuba is
# PE-paced beyond x chunk 0, so there is no DMA idle to fill with g0).
SCHED = ([("pair", "f", k) for k in range(6)] + [("solo", "f", None)]
         + [("pair", "g", k) for k in range(6)] + [("solo", "g", None)])

# Pairs are either KARATSUBA (6 weight tiles [A0,A1,Qn0,Qn1,P0,P1]) or
# PLAIN 4-tap (4 tiles [V0,V1,V2,V3]). Plain: both pair-0s (blk-2-strided
# y-even writes fall off the DVE fast path; plain needs no D at all, which
# also declutters the DMA-paced startup), both pair-5s (karatsuba aux
# overloads ACT/DVE in the tail) and g-pair0's merge with solo-f.
PLAIN = {("f", 0), ("g", 0), ("f", 5), ("g", 5)}
WBASE = {}
_w = 0
for _e in SCHED:
    WBASE[(_e[0], _e[1], _e[2])] = _w
    if _e[0] == "pair":
        _w += 4 if (_e[1], _e[2]) in PLAIN else 6
    else:
        _w += 2
NW = _w                     # 70

W_HEAD = 4                  # f-pair0's V0..V3: DMA'd before x chunk 0
W_MID = 16                  # through f2's weights

# x input chunks: fine early (first tiles start sooner), coarse late
XEDGE = [0, 512, 1024, 2048, 3072, 4096, 6144, 8192]
# output chunks, big early / small late so the final transfer is tiny
QEDGE = [0, 2048, 4096, 6144, 7168, 7680, 7936, 8192]
NQ = len(QEDGE) - 1

_PROGRAM_CACHE = {}


def _build_program():
    import concourse.bacc as bacc
    import concourse.tile as tile
    from concourse import mybir
    from concourse.ap import AP

    f32 = mybir.dt.float32
    fp16 = mybir.dt.float16
    AF = mybir.ActivationFunctionType

    nc = bacc.Bacc("TRN2", target_bir_lowering=False, debug=False)

    consts = nc.dram_tensor("consts", [P, 2 * P + 1], f32,
                            kind="ExternalInput").ap()
    xin = nc.dram_tensor("xr", [P, T], fp16, kind="ExternalInput").ap()
    wconv = nc.dram_tensor("wr", [P, NW * P], fp16, kind="ExternalInput").ap()
    outs = [nc.dram_tensor(f"out_q{q}", [P, QEDGE[q + 1] - QEDGE[q]], fp16,
                           kind="ExternalOutput").ap() for q in range(NQ)]

    with tile.TileContext(nc) as tc:
        with (
            tc.tile_pool(name="persist", bufs=1) as persist,
            tc.tile_pool(name="epool", bufs=4) as epool,
            tc.tile_pool(name="qpool", bufs=3) as qpool,
            tc.tile_pool(name="mpsum", bufs=6, space="PSUM") as mpsum,
            tc.tile_pool(name="p5psum", bufs=2, space="PSUM") as p5psum,
        ):
            # ---- persistent buffers ---------------------------------------
            x0 = persist.tile([P, MARGIN + T], fp16, name="x0", tag="x0")
            bufA = persist.tile([P, MARGIN + T], fp16, name="bufA", tag="bufA")
            bufB = persist.tile([P, MARGIN + T], fp16, name="bufB", tag="bufB")
            scratch = persist.tile([P, MARGIN + T], fp16, name="scratch",
                                   tag="scratch")
            D0 = persist.tile([P, MARGIN + T], fp16, name="D0", tag="D0")
            D1 = persist.tile([P, MARGIN + T], fp16, name="D1", tag="D1")
            D2 = persist.tile([P, MARGIN + T], fp16, name="D2", tag="D2")
            call = persist.tile([P, 2 * P + 1], f32, name="call", tag="call")
            wall = persist.tile([P, NW * P], fp16, name="wall", tag="wall")
            tanh16 = persist.tile([P, T], fp16, name="tanh16", tag="tanh16")
            out16 = persist.tile([P, T], fp16, name="out16", tag="out16")
            # only x0's margin gates the first matmul; the rest memset
            # on the otherwise-idle GpSimd so the DVE queue starts clear
            nc.vector.memset(x0[:, 0:MARGIN], 0.0)
            for buf in (bufA, bufB, scratch, D0, D1, D2):
                nc.gpsimd.memset(buf[:, 0:MARGIN], 0.0)

            # ---- input DMAs, ordered by first consumption -----------------
            nc.sync.dma_start(out=wall[:, 0:P], in_=wconv[:, 0:P])
            nc.sync.dma_start(
                out=x0[:, MARGIN + XEDGE[0]:MARGIN + XEDGE[1]],
                in_=xin[:, XEDGE[0]:XEDGE[1]])
            nc.sync.dma_start(out=wall[:, P:W_HEAD * P],
                              in_=wconv[:, P:W_HEAD * P])
            nc.sync.dma_start(out=call, in_=consts)
            # x chunks 1-2 land before f1/f2's weight block (startup is
            # DMA-bandwidth-paced; f1's weights aren't needed until ~25us)
            for xc in range(1, 3):
                nc.sync.dma_start(
                    out=x0[:, MARGIN + XEDGE[xc]:MARGIN + XEDGE[xc + 1]],
                    in_=xin[:, XEDGE[xc]:XEDGE[xc + 1]])
            nc.sync.dma_start(out=wall[:, W_HEAD * P:W_MID * P],
                              in_=wconv[:, W_HEAD * P:W_MID * P])
            for xc in range(3, len(XEDGE) - 1):
                nc.sync.dma_start(
                    out=x0[:, MARGIN + XEDGE[xc]:MARGIN + XEDGE[xc + 1]],
                    in_=xin[:, XEDGE[xc]:XEDGE[xc + 1]])
            nc.sync.dma_start(out=wall[:, W_MID * P:],
                              in_=wconv[:, W_MID * P:])

            # ---- PE lane warm-ups + h @ V biases --------------------------
            bias_ps = mpsum.tile([P, 8], f32, name="bias_ps", tag="mp")
            x0w = x0[:, MARGIN:MARGIN + 1]
            ww0 = wall[:, 0:1]
            ww1 = wall[:, W_HEAD * P:W_HEAD * P + 1]
            ww2 = wall[:, W_MID * P:W_MID * P + 1]
            nc.tensor.matmul(bias_ps[0:1, 4:5], lhsT=x0w, rhs=x0w,
                             start=True, stop=True)
            nc.tensor.matmul(bias_ps[0:1, 5:6], lhsT=ww0, rhs=ww0,
                             start=True, stop=True)
            nc.tensor.matmul(bias_ps[0:1, 6:7], lhsT=ww1, rhs=ww1,
                             start=True, stop=True)
            nc.tensor.matmul(bias_ps[0:1, 7:8], lhsT=ww2, rhs=ww2,
                             start=True, stop=True)
            h_t = call[:, 2 * P:2 * P + 1]
            bias = []
            for i in range(2):
                nc.tensor.matmul(bias_ps[:, i:i + 1],
                                 lhsT=call[:, i * P:(i + 1) * P],
                                 rhs=h_t, start=True, stop=True)
                bias_sb = persist.tile([P, 1], f32, name=f"bias{i}",
                                       tag=f"bias{i}")
                nc.vector.tensor_copy(bias_sb, bias_ps[:, i:i + 1])
                bias.append(bias_sb)

            # ---- helpers --------------------------------------------------
            def sap(buf, col, blkstride, nblk, blklen):
                """Strided AP: nblk blocks of blklen cols every blkstride."""
                base = buf[:, col:col + 1]
                return AP(base.tensor, base.offset,
                          [list(base.ap[0]), [blkstride, nblk], [1, blklen]])

            def wt(base, i):
                return wall[:, (base + i) * P:(base + i + 1) * P]

            ectr = [0]

            def emit_D(db, src, k, s):
                """D2 for pair k over supertile s into db, alternating
                DVE / GpSimd. Pair 4: tile 0 of supertile 0 is a copy
                (x(t-512) is beyond the margin)."""
                twod = 2 * PAIR_D[k]
                c0 = s * ST
                # 50:50 DVE:GpSimd — DVE is the k-phase bottleneck
                # (ye + D + yo share), GpSimd idles; its 2.2us latency is
                # hidden by the one-pair prefetch lead
                eng = nc.gpsimd if ectr[0] % 2 == 1 else nc.vector
                ectr[0] += 1
                dst = db[:, MARGIN + c0:MARGIN + c0 + ST]
                if twod <= MARGIN:
                    eng.tensor_sub(dst, src[:, MARGIN + c0:MARGIN + c0 + ST],
                                   src[:, MARGIN + c0 - twod:
                                       MARGIN + c0 + ST - twod])
                else:  # pair 4, twod = 512
                    if s == 0:
                        eng.tensor_copy(db[:, MARGIN:MARGIN + TILE],
                                        src[:, MARGIN:MARGIN + TILE])
                        eng.tensor_sub(
                            db[:, MARGIN + TILE:MARGIN + ST],
                            src[:, MARGIN + TILE:MARGIN + ST],
                            src[:, MARGIN:MARGIN + TILE])
                    else:
                        eng.tensor_sub(dst,
                                       src[:, MARGIN + c0:MARGIN + c0 + ST],
                                       src[:, MARGIN + c0 - twod:
                                           MARGIN + c0 + ST - twod])

            # y-odd copies: 3:1 ACT:DVE (DVE also carries ye TTs + half of D)
            yo_rr = [0]

            def yo_drain():
                i = yo_rr[0] % 4
                yo_rr[0] += 1
                return nc.vector.tensor_copy if i == 3 else nc.scalar.copy

            # plain-pair drains: 1:1 ACT:DVE
            pl_rr = [0]

            def pl_drain():
                pl_rr[0] += 1
                return (nc.scalar.copy if pl_rr[0] % 2 == 0
                        else nc.vector.tensor_copy)

            def pair_pq(br, k, s, src, dst, db, wb):
                """p and q matmuls + y-even for supertile s; returns pp."""
                d = PAIR_D[k]
                twod, fourd = 2 * d, 4 * d
                c0 = s * ST
                nb = ST // fourd

                def ev(buf, off):
                    return sap(buf, MARGIN + c0 - off, fourd, nb, twod)

                pp = mpsum.tile([P, ST // 2], f32, name=f"pp_{br}{k}_{s}",
                                tag="mp")
                pq = mpsum.tile([P, ST // 2], f32, name=f"pq_{br}{k}_{s}",
                                tag="mp")
                nc.tensor.matmul(pp, lhsT=wt(wb, 0), rhs=ev(src, 0),
                                 start=True, stop=False)
                nc.tensor.matmul(pp, lhsT=wt(wb, 1), rhs=ev(src, d),
                                 start=False, stop=True)
                nc.tensor.matmul(pq, lhsT=wt(wb, 2), rhs=ev(db, 0),
                                 start=True, stop=False)
                nc.tensor.matmul(pq, lhsT=wt(wb, 3), rhs=ev(db, d),
                                 start=False, stop=True)
                # verifier forbids dual-PSUM TT: bounce q through SBUF,
                # then y-even = pp(PSUM) + q16(SBUF) on DVE; pp survives for
                # the r accumulation.
                q16 = qpool.tile([P, ST // 2], fp16, name=f"q16_{br}{k}_{s}",
                                 tag="q16")
                nc.scalar.copy(q16, pq)
                nc.vector.tensor_add(ev(dst, 0), pp, q16)
                return pp

            def pair_r(br, k, s, dst, db, wb, pp):
                """r matmuls accumulate into pp; y-odd copy."""
                d = PAIR_D[k]
                twod, fourd = 2 * d, 4 * d
                c0 = s * ST
                nb = ST // fourd

                def ev(buf, off):
                    return sap(buf, MARGIN + c0 - off, fourd, nb, twod)

                nc.tensor.matmul(pp, lhsT=wt(wb, 4), rhs=ev(db, -twod),
                                 start=False, stop=False,
                                 skip_group_check=True)
                nc.tensor.matmul(pp, lhsT=wt(wb, 5), rhs=ev(db, -twod + d),
                                 start=False, stop=True,
                                 skip_group_check=True)
                yo_drain()(ev(dst, -twod), pp)

            def plain_tile(br, k, j, src, dst, wb, drain_eng=None,
                           fine=False):
                """PLAIN 4-tap composite pair at 512-col tile granularity.
                Tap i reads src(t - i*d); margin covers small d, tile-aligned
                skips cover d=1024 (skip while (j+1)*TILE <= i*d)."""
                d = PAIR_D[k]
                c0 = j * TILE
                taps = [i for i in range(4) if (j + 1) * TILE > i * d]
                ps = p5psum.tile([P, TILE], f32, name=f"pl_{br}{k}_{j}",
                                 tag="p5")
                for n, i in enumerate(taps):
                    nc.tensor.matmul(
                        ps, lhsT=wt(wb, i),
                        rhs=src[:, MARGIN + c0 - i * d:
                                MARGIN + c0 - i * d + TILE],
                        start=(n == 0), stop=(n == len(taps) - 1))
                eng = drain_eng if drain_eng is not None else pl_drain()
                if fine:
                    eng(dst[:, MARGIN + c0:MARGIN + c0 + 256], ps[:, 0:256])
                    eng(dst[:, MARGIN + c0 + 256:MARGIN + c0 + TILE],
                        ps[:, 256:TILE])
                else:
                    eng(dst[:, MARGIN + c0:MARGIN + c0 + TILE], ps)

            def solo_tile(br, j, src, wb, fuse):
                """Solo layer 12 (d=4096): psum -> fuse(j, ps)."""
                c0 = j * TILE
                ps = mpsum.tile([P, TILE], f32, name=f"s12{br}_{j}", tag="mp")
                has0 = c0 >= SOLO_D
                nc.tensor.matmul(ps, lhsT=wt(wb, 0),
                                 rhs=src[:, MARGIN + c0:MARGIN + c0 + TILE],
                                 start=True, stop=not has0)
                if has0:
                    nc.tensor.matmul(
                        ps, lhsT=wt(wb, 1),
                        rhs=src[:, MARGIN + c0 - SOLO_D:
                                MARGIN + c0 - SOLO_D + TILE],
                        start=False, stop=True)
                fuse(j, ps)

            def tanh_drain(j, ps):
                t0 = j * TILE
                nc.scalar.activation(tanh16[:, t0:t0 + TILE], ps,
                                     AF.Tanh, bias=bias[0])

            nxt_q = [0]

            def flush_outputs(jj):
                while (nxt_q[0] < NQ
                       and QEDGE[nxt_q[0] + 1] <= (jj + 1) * TILE):
                    q = nxt_q[0]
                    nc.sync.dma_start(
                        out=outs[q], in_=out16[:, QEDGE[q]:QEDGE[q + 1]])
                    nxt_q[0] += 1

            def epilogue(j, ps):
                t0 = j * TILE
                sig = epool.tile([P, TILE], fp16, name=f"sig{j}", tag="sig")
                nc.scalar.activation(sig, ps, AF.Sigmoid, bias=bias[1])
                # alternate the gating mul DVE/GpSimd: the tail is aux-bound
                eng = nc.vector if j % 2 == 0 else nc.gpsimd
                eng.tensor_mul(out16[:, t0:t0 + TILE],
                               tanh16[:, t0:t0 + TILE], sig)
                flush_outputs(j)

            # ---- plan -----------------------------------------------------
            # chains: f: x0->A->B->A->B->A->B ->tanh16; g: x0->scratch->A->..
            fchain = [x0, bufA, bufB, bufA, bufB, bufA, bufB]
            gchain = [x0, scratch, bufA, bufB, bufA, bufB, bufA]
            plan = []
            for k in range(6):
                plan.append(("pair", "f", k, fchain[k], fchain[k + 1]))
            plan.append(("solo", "f", None, fchain[6], None))
            for k in range(6):
                plan.append(("pair", "g", k, gchain[k], gchain[k + 1]))
            plan.append(("solo", "g", None, gchain[6], None))

            # D buffers ping-pong over karatsuba pairs in plan order; each
            # pair's D is prefetched during the PREVIOUS plan entry (g1's
            # during the merged solo-f + g0 phase).
            kpairs = [e for e in plan
                      if e[0] == "pair" and (e[1], e[2]) not in PLAIN]
            dbuf_of = {}
            for n, e in enumerate(kpairs):
                dbuf_of[(e[1], e[2])] = (D0, D1, D2)[n % 3]

            def prefetch_D(i, s):
                """During plan entry i, emit supertile s of the next
                karatsuba pair's D (if entry i+1 is one)."""
                if i + 1 < len(plan):
                    e = plan[i + 1]
                    if e[0] == "pair" and (e[1], e[2]) not in PLAIN:
                        emit_D(dbuf_of[(e[1], e[2])], e[3], e[2], s)

            for i, e in enumerate(plan):
                kind, br, k, src, dst = e
                if kind == "pair" and (br, k) not in PLAIN:
                    db = dbuf_of[(br, k)]
                    wb = WBASE[("pair", br, k)]
                    prev = None
                    for s in range(NST):
                        pp = pair_pq(br, k, s, src, dst, db, wb)
                        if prev is not None:
                            pair_r(br, k, s - 1, dst, db, wb, prev)
                            prefetch_D(i, s - 1)
                        prev = pp
                    pair_r(br, k, NST - 1, dst, db, wb, prev)
                    prefetch_D(i, NST - 1)
                elif kind == "pair" and not (br == "g" and k == 0):
                    # plain pairs run standalone (g-pair0 merges with solo-f)
                    wb = WBASE[("pair", br, k)]
                    if br == "g":
                        # tail: interleave solo-g + epilogue per tile
                        wbs = WBASE[("solo", "g", None)]
                        for j in range(NT):
                            plain_tile(br, 5, j, src, dst, wb,
                                       drain_eng=nc.vector.tensor_copy,
                                       fine=(j == NT - 1))
                            if j >= 1:
                                solo_tile("g", j - 1, dst, wbs, epilogue)
                        # last tile: 256-col chains so the serial suffix
                        # (mm -> sigmoid -> mul -> DMA) halves
                        jf = NT - 1
                        for h in (0, 1):
                            o = jf * TILE + h * 256
                            psf = mpsum.tile([P, 256], f32,
                                             name=f"s12gf{h}", tag="mp")
                            nc.tensor.matmul(
                                psf, lhsT=wt(wbs, 0),
                                rhs=dst[:, MARGIN + o:MARGIN + o + 256],
                                start=True, stop=False)
                            nc.tensor.matmul(
                                psf, lhsT=wt(wbs, 1),
                                rhs=dst[:, MARGIN + o - SOLO_D:
                                        MARGIN + o - SOLO_D + 256],
                                start=False, stop=True)
                            sigf = epool.tile([P, 256], fp16,
                                              name=f"sigf{h}", tag="sig")
                            nc.scalar.activation(sigf, psf, AF.Sigmoid,
                                                 bias=bias[1])
                            nc.vector.tensor_mul(out16[:, o:o + 256],
                                                 tanh16[:, o:o + 256], sigf)
                            nc.sync.dma_start(out=outs[NQ - 2 + h],
                                              in_=out16[:, o:o + 256])
                    else:
                        for j in range(NT):
                            plain_tile(br, k, j, src, dst, wb)
                            if j % 2 == 1:
                                prefetch_D(i, j // 2)
                elif kind == "solo" and br == "f":
                    # merged phase: solo-f's tanh drains own ACT, so the
                    # PE-heavy plain g-pair0 interleaves here (g0's drains
                    # forced to DVE); prefetches g1's D from scratch.
                    wb_s = WBASE[("solo", "f", None)]
                    eg = plan[i + 1]
                    assert eg[:3] == ("pair", "g", 0)
                    wb_g0 = WBASE[("pair", "g", 0)]
                    for j in range(NT):
                        solo_tile("f", j, src, wb_s, tanh_drain)
                        plain_tile("g", 0, j, eg[3], eg[4], wb_g0,
                                   drain_eng=nc.vector.tensor_copy)
                        if j % 2 == 1:
                            prefetch_D(i + 1, j // 2)
                # g-pair0 handled in the merged phase; solo-g in the tail

    nc.compile()
    return nc


def get_program():
    if "nc" not in _PROGRAM_CACHE:
        _PROGRAM_CACHE["nc"] = _build_program()
    return _PROGRAM_CACHE["nc"]


def make_in_maps(x, h, Wf, Wg, Vf, Vg):
    x = np.asarray(x, dtype=np.float32)
    h = np.asarray(h, dtype=np.float32)
    eye2 = np.eye(2, dtype=np.float64)
    Wn = {"f": np.asarray(Wf, dtype=np.float64),
          "g": np.asarray(Wg, dtype=np.float64)}
    wpack = np.zeros((NW, P, P), dtype=np.float32)
    for (kind, br, k), base in WBASE.items():
        Wb = Wn[br]
        if kind == "pair":
            W0a, W1a = Wb[2 * k, 0], Wb[2 * k, 1]
            W0b, W1b = Wb[2 * k + 1, 0], Wb[2 * k + 1, 1]
            V0 = W1a @ W1b
            V1 = W0a @ W1b
            V2 = W1a @ W0b
            V3 = W0a @ W0b
            if (br, k) in PLAIN:    # plain 4-tap
                mats = [V0, V1, V2, V3]
            else:                   # karatsuba
                mats = [V0 + V2, V1 + V3, -V2, -V3, V0, V1]
        else:
            mats = [Wb[12, 1], Wb[12, 0]]
        for i, m in enumerate(mats):
            wpack[base + i] = np.kron(eye2, m).astype(np.float32)
    wcols = wpack.transpose(1, 0, 2).reshape(P, NW * P).astype(np.float16)
    vcat = np.concatenate(
        [np.kron(np.eye(2, dtype=np.float32), np.asarray(V, dtype=np.float32))
         for V in (Vf, Vg)], axis=1)  # [128, 256]

    in_maps = []
    for core in range(N_CORES):
        sl = slice(core * BPC, (core + 1) * BPC)
        xcm = x[sl].transpose(0, 2, 1).reshape(P, T) \
            .astype(np.float16)  # [(b,c), t]
        consts = np.ascontiguousarray(
            np.concatenate([vcat, h[sl].reshape(P, 1)], axis=1))
        in_maps.append({"consts": consts, "xr": xcm, "wr": wcols})
    return in_maps


def _to_f32(a):
    a = np.asarray(a)
    if a.dtype in (np.float32, np.float16):
        return a.astype(np.float32)
    u = a.view(np.uint16).astype(np.uint32) << np.uint32(16)
    return u.view(np.float32)


def assemble_output(results):
    full = np.empty((B, T, C), dtype=np.float32)
    for core, r in enumerate(results):
        cm = np.concatenate(
            [_to_f32(r[f"out_q{q}"]) for q in range(NQ)], axis=1)
        full[core * BPC:(core + 1) * BPC] = \
            cm.reshape(BPC, C, T).transpose(0, 2, 1)
    return full


def kernel(x, h, Wf, Wg, Vf, Vg):
    from concourse import bass_utils

    nc = get_program()
    in_maps = make_in_maps(x, h, Wf, Wg, Vf, Vg)
    res = bass_utils.run_bass_kernel_spmd(nc, in_maps,
                                          core_ids=list(range(N_CORES)))
    return assemble_output(res.results)
